# revision 16
# baseline (speedup 1.0000x reference)
"""Trainium2 Bass kernel for CognitionNetwork (GNN message passing + LSTM attention).

Contract: kernel(**inputs) takes FULL inputs, returns FULL [2048, 400] q_star.
Internally shards 2048 conversations (segments) contiguously across 8 NeuronCores
(256 segments each). Nodes are re-laid-out on the host so that each block of 32
segments owns a fixed number of 128-node tiles -> the SPMD device program is
fully static; all data-dependence lives in host-built indicator matrices.

Math notes:
  - segment softmax is computed WITHOUT max subtraction (scores are bounded,
    |e| < ~25 for this distribution; exp stays well inside f32 range).
  - softmax denominator comes for free from an appended ones-column on x.
  - LSTM bias is folded into the weight matrix via a ones-row on the input.
  - All matmuls run as float32r (full PE speed at N>=256, near-fp32 accuracy).
"""

import os
from contextlib import ExitStack

import numpy as np

import concourse.bass as bass
import concourse.bacc as bacc
import concourse.tile as tile
from concourse import masks, mybir
from concourse.bass_utils import run_bass_kernel_spmd

CORES = 8
B = 2048
F = 200
FW = F + 1            # x tile width: 200 features + ones column
SEG_PER_CORE = B // CORES   # 256
BS = 32               # segments per block
BLOCKS = SEG_PER_CORE // BS  # 8
STEPS = 3

TRACE = bool(int(os.environ.get("KERNEL_TRACE", "0")))
LAST_RESULT = None
_PROG_CACHE = {}


def _build_program(T_pad: int, nsteps: int = STEPS) -> bass.Bass:
    NT = BLOCKS * T_pad          # node tiles per core
    XW = NT * FW + 56            # packed x width (+56 so 256-wide reads never overrun)

    nc = bacc.Bacc("TRN2", target_bir_lowering=False, debug=False)
    f32 = mybir.dt.float32
    f32r = mybir.dt.float32r
    AF = mybir.ActivationFunctionType
    OP = mybir.AluOpType

    xt_d = nc.dram_tensor("xt", [128, XW], f32, kind="ExternalInput").ap()
    w_d = nc.dram_tensor("w", [128, NT * BS], f32, kind="ExternalInput").ap()
    wt_d = nc.dram_tensor("wt", [128, 2 * T_pad * 128], f32, kind="ExternalInput").ap()
    cosp_d = nc.dram_tensor("cosp", [128, NT], f32, kind="ExternalInput").ap()
    qs0t_d = nc.dram_tensor("qs0t", [401, 256], f32, kind="ExternalInput").ap()
    ident_d = nc.dram_tensor("ident", [128, 128], f32, kind="ExternalInput").ap()
    zro_d = nc.dram_tensor("zro", [128, 256], f32, kind="ExternalInput").ap()
    wbig_d = nc.dram_tensor("wbig", [617, 800], f32, kind="ExternalInput").ap()
    wc_d = nc.dram_tensor("wc", [424, 800], f32, kind="ExternalInput").ap()
    qout_d = nc.dram_tensor("qout", [256, 400], f32, kind="ExternalOutput").ap()

    with tile.TileContext(nc) as tc:
        with ExitStack() as ctx:
            res = ctx.enter_context(tc.tile_pool(name="res", bufs=1))
            state = ctx.enter_context(tc.tile_pool(name="state", bufs=1))
            prodp = ctx.enter_context(tc.tile_pool(name="prodp", bufs=3))
            exwp = ctx.enter_context(tc.tile_pool(name="exwp", bufs=6))
            ebp = ctx.enter_context(tc.tile_pool(name="ebp", bufs=2))
            sbt = ctx.enter_context(tc.tile_pool(name="sbt", bufs=2))
            inp = ctx.enter_context(tc.tile_pool(name="inp", bufs=2))
            psA = ctx.enter_context(tc.tile_pool(name="psA", bufs=2, space="PSUM"))
            psB = ctx.enter_context(tc.tile_pool(name="psB", bufs=2, space="PSUM"))
            psC = ctx.enter_context(tc.tile_pool(name="psC", bufs=2, space="PSUM"))
            psT = ctx.enter_context(tc.tile_pool(name="psT", bufs=2, space="PSUM"))

            # ---------------- resident loads ----------------
            identity = res.tile([128, 128], f32r)
            nc.gpsimd.dma_start(identity[:], ident_d[:])

            xt_sb = res.tile([128, XW], f32r)
            CW = T_pad * FW
            for g in range(BLOCKS):
                lo = g * CW
                hi = (g + 1) * CW if g < BLOCKS - 1 else XW
                nc.gpsimd.dma_start(xt_sb[:, lo:hi], xt_d[:, lo:hi])

            w_sb = res.tile([128, NT * BS], f32)
            nc.sync.dma_start(w_sb[:], w_d[:])
            wt_sb = res.tile([128, 2 * T_pad * 128], f32r)
            nc.gpsimd.dma_start(wt_sb[:], wt_d[:])
            cosp_sb = res.tile([128, NT], f32)
            nc.sync.dma_start(cosp_sb[:], cosp_d[:])

            # LSTM weights, step 1: [W_ih^T; zeros16; W_hh^T] rows + bias row
            wmat = []
            ksz = [128, 128, 128, 128, 104]
            koff = [0, 128, 256, 384, 512]
            for k, o in zip(ksz, koff):
                t = res.tile([k, 800], f32r, tag=f"wm{o}", name=f"wm{o}")
                nc.gpsimd.dma_start(t[:], wbig_d[o : o + k, :])
                wmat.append(t)
            wbias = res.tile([1, 800], f32r)
            nc.gpsimd.dma_start(wbias[:], wbig_d[616:617, :])
            ones_c = res.tile([1, 256], f32r)
            nc.gpsimd.dma_start(ones_c[:], qs0t_d[400:401, :])

            # h, c, r state (seg-layout, two 128-partition halves)
            h_sb = [state.tile([128, 256], f32r, tag=f"h{i}", name=f"h{i}") for i in range(2)]
            c_sb = [state.tile([128, F], f32, tag=f"c{i}", name=f"c{i}") for i in range(2)]
            r_sb = [state.tile([128, F], f32r, tag=f"r{i}", name=f"r{i}") for i in range(2)]
            for i in range(2):
                nc.gpsimd.dma_start(h_sb[i][:], zro_d[:])
                nc.vector.memset(c_sb[i][:], 0.0)

            # ---------------- phase 0: h0 = segment_sum(cos * x) ----------------
            for g in range(BLOCKS):
                h0ps = psB.tile([32, 256], f32, tag="rblk")
                for i in range(T_pad):
                    t = g * T_pad + i
                    cw = exwp.tile([128, BS], f32r, tag="exw")
                    nc.vector.tensor_scalar_mul(
                        cw[:], w_sb[:, t * BS : (t + 1) * BS], cosp_sb[:, t : t + 1]
                    )
                    nc.tensor.matmul(
                        h0ps[:],
                        lhsT=cw[:],
                        rhs=xt_sb[:, t * FW : t * FW + 256],
                        start=(i == 0),
                        stop=(i == T_pad - 1),
                    )
                dst = h_sb[g // 4]
                p0 = 32 * (g % 4)
                nc.vector.tensor_copy(dst[p0 : p0 + 32, 0:F], h0ps[:, 0:F])

            # ---------------- steps ----------------
            for s in range(nsteps):
                # ---- build transposed LSTM input chunks ----
                # SBUF compute APs must start at partition 0/32/64/96 (with
                # counts <=128/32/64/32), so chunk sections sit at 32-aligned
                # offsets with zero padding; PSUM sources are unrestricted.
                if s == 0:
                    # input rows = [q_star0 (400); zeros (16); h0 (200)] + ones
                    # A3 = [qs0[384:400]; pad16; h0T rows 0:96]
                    # A4 = [h0T rows 96:200]  (104 rows)
                    A0 = inp.tile([128, 256], f32r, tag="B0")
                    A1 = inp.tile([128, 256], f32r, tag="B1")
                    A2 = inp.tile([128, 256], f32r, tag="B2")
                    A3 = inp.tile([128, 256], f32r, tag="A3")
                    A4 = inp.tile([104, 256], f32r, tag="A4")
                    nc.gpsimd.dma_start(A3[0:32, :], zro_d[0:32, :])
                    nc.gpsimd.dma_start(A0[:], qs0t_d[0:128, :])
                    nc.gpsimd.dma_start(A1[:], qs0t_d[128:256, :])
                    nc.gpsimd.dma_start(A2[:], qs0t_d[256:384, :])
                    nc.gpsimd.dma_start(A3[0:16, :], qs0t_d[384:400, :])
                    for half in range(2):
                        src = h_sb[half]
                        co = 128 * half
                        th = psT.tile([128, 128], f32r, tag="tp")
                        nc.tensor.transpose(th[:], src[:, 0:128], identity[:])
                        nc.vector.tensor_copy(A3[32:64, co : co + 128], th[0:32, :])
                        nc.vector.tensor_copy(A3[64:96, co : co + 128], th[32:64, :])
                        nc.vector.tensor_copy(A3[96:128, co : co + 128], th[64:96, :])
                        nc.vector.tensor_copy(A4[0:32, co : co + 128], th[96:128, :])
                        tl = psT.tile([72, 128], f32r, tag="tp")
                        nc.tensor.transpose(tl[:], src[:, 128:200], identity[:])
                        nc.vector.tensor_copy(A4[32:64, co : co + 128], tl[0:32, :])
                        nc.vector.tensor_copy(A4[64:96, co : co + 128], tl[32:64, :])
                        nc.vector.tensor_copy(A4[96:104, co : co + 128], tl[64:72, :])
                    chunks = [(A0, 128), (A1, 128), (A2, 128), (A3, 128), (A4, 104), (ones_c, 1)]
                    wtiles = wmat + [wbias]
                else:
                    # input rows = [h (200); zeros (24); r (200)] + ones
                    # C1 = [hT 128:200; pad24; rT 0:32], C2 = rT 32:160, C3 = rT 160:200
                    B0 = inp.tile([128, 256], f32r, tag="B0")
                    B1 = inp.tile([128, 256], f32r, tag="B1")
                    B2 = inp.tile([128, 256], f32r, tag="B2")
                    B3 = inp.tile([40, 256], f32r, tag="B3")
                    nc.gpsimd.dma_start(B1[64:96, :], zro_d[0:32, :])
                    for half in range(2):
                        hs = h_sb[half]
                        rs = r_sb[half]
                        co = 128 * half
                        th = psT.tile([128, 128], f32r, tag="tp")
                        nc.tensor.transpose(th[:], hs[:, 0:128], identity[:])
                        nc.vector.tensor_copy(B0[:, co : co + 128], th[:])
                        tl = psT.tile([72, 128], f32r, tag="tp")
                        nc.tensor.transpose(tl[:], hs[:, 128:200], identity[:])
                        nc.vector.tensor_copy(B1[0:72, co : co + 128], tl[:])
                        tr = psT.tile([128, 128], f32r, tag="tp")
                        nc.tensor.transpose(tr[:], rs[:, 0:128], identity[:])
                        nc.vector.tensor_copy(B1[96:128, co : co + 128], tr[0:32, :])
                        nc.vector.tensor_copy(B2[0:32, co : co + 128], tr[32:64, :])
                        nc.vector.tensor_copy(B2[32:64, co : co + 128], tr[64:96, :])
                        nc.vector.tensor_copy(B2[64:96, co : co + 128], tr[96:128, :])
                        tq = psT.tile([72, 128], f32r, tag="tp")
                        nc.tensor.transpose(tq[:], rs[:, 128:200], identity[:])
                        nc.vector.tensor_copy(B2[96:128, co : co + 128], tq[0:32, :])
                        nc.vector.tensor_copy(B3[0:32, co : co + 128], tq[32:64, :])
                        nc.vector.tensor_copy(B3[32:40, co : co + 128], tq[64:72, :])
                    chunks = [(B0, 128), (B1, 128), (B2, 128), (B3, 40), (ones_c, 1)]
                    wtiles = wmat[:4] + [wbias]  # wmat reloaded with wc after step 1

                # ---- gates + cell update, per 128-segment half ----
                for half in range(2):
                    co = 128 * half
                    acts = {}
                    for part in range(2):  # part 0 -> i|f gates, part 1 -> g|o
                        ps = psC.tile([128, 400], f32, tag="gates")
                        nch = len(chunks)
                        for ci, (ctile, kdim) in enumerate(chunks):
                            nc.tensor.matmul(
                                ps[:],
                                lhsT=ctile[0:kdim, co : co + 128],
                                rhs=wtiles[ci][0:kdim, 400 * part : 400 * part + 400],
                                start=(ci == 0),
                                stop=(ci == nch - 1),
                            )
                        if part == 0:
                            si = sbt.tile([128, F], f32, tag="si")
                            nc.scalar.activation(si[:], ps[:, 0:F], AF.Sigmoid)
                            sf = sbt.tile([128, F], f32, tag="sf")
                            nc.scalar.activation(sf[:], ps[:, F:400], AF.Sigmoid)
                            acts["i"], acts["f"] = si, sf
                        else:
                            tg = sbt.tile([128, F], f32, tag="tg")
                            nc.scalar.activation(tg[:], ps[:, 0:F], AF.Tanh)
                            so = sbt.tile([128, F], f32, tag="so")
                            nc.scalar.activation(so[:], ps[:, F:400], AF.Sigmoid)
                            acts["g"], acts["o"] = tg, so
                    # c = sigm(f)*c + sigm(i)*tanh(g);  h = sigm(o)*tanh(c)
                    ch = c_sb[half]
                    tmp = sbt.tile([128, F], f32, tag="tmp")
                    nc.vector.tensor_mul(tmp[:], acts["f"][:], ch[:])
                    nc.vector.tensor_mul(ch[:], acts["i"][:], acts["g"][:])
                    nc.vector.tensor_add(ch[:], tmp[:], ch[:])
                    tct = sbt.tile([128, F], f32, tag="tct")
                    nc.scalar.activation(tct[:], ch[:], AF.Tanh)
                    nc.vector.tensor_mul(h_sb[half][:, 0:F], acts["o"][:], tct[:])

                # step 1 only: swap in the combined weights for steps 2..3
                if s == 0:
                    for k, o, t in zip([128, 128, 128, 40], [0, 128, 256, 384], wmat[:4]):
                        nc.gpsimd.dma_start(t[0:k, :], wc_d[o : o + k, :])

                # ---- attention: e, softmax, r ----
                for g in range(BLOCKS):
                    lane = g % 4
                    p0 = 32 * lane
                    qt = h_sb[g // 4]
                    eb = ebp.tile([128, T_pad], f32, tag="eb")
                    exb = ebp.tile([128, T_pad], f32, tag="exb")
                    for i in range(T_pad):
                        t = g * T_pad + i
                        slot = (g // 4) * T_pad + i
                        qg = psA.tile([128, 256], f32, tag="qg")
                        nc.tensor.matmul(
                            qg[:],
                            lhsT=wt_sb[p0 : p0 + 32, 128 * slot : 128 * slot + 128],
                            rhs=qt[p0 : p0 + 32, 0:256],
                            start=True,
                            stop=True,
                            tile_position=(p0, 0),
                        )
                        prod = prodp.tile([128, F], f32, tag="prod")
                        nc.vector.scalar_tensor_tensor(
                            out=prod[:],
                            in0=xt_sb[:, t * FW : t * FW + F].bitcast(f32),
                            scalar=1.0,
                            in1=qg[:, 0:F],
                            op0=OP.mult,
                            op1=OP.mult,
                            accum_out=eb[:, i : i + 1],
                        )
                    nc.scalar.activation(exb[:], eb[:], AF.Exp)
                    rps = psB.tile([32, 256], f32, tag="rblk")
                    for i in range(T_pad):
                        t = g * T_pad + i
                        exw = exwp.tile([128, BS], f32r, tag="exw")
                        nc.vector.tensor_scalar_mul(
                            exw[:], w_sb[:, t * BS : (t + 1) * BS], exb[:, i : i + 1]
                        )
                        nc.tensor.matmul(
                            rps[:],
                            lhsT=exw[:],
                            rhs=xt_sb[:, t * FW : t * FW + 256],
                            start=(i == 0),
                            stop=(i == T_pad - 1),
                        )
                    dinv = sbt.tile([32, 1], f32, tag="dinv")
                    nc.vector.reciprocal(dinv[:], rps[:, F : F + 1])
                    rdst = r_sb[g // 4]
                    nc.vector.tensor_scalar_mul(
                        rdst[p0 : p0 + 32, 0:F], rps[:, 0:F], dinv[:]
                    )

            # ---------------- output: q_star = [h | r] ----------------
            for half in range(2):
                ro = 128 * half
                nc.sync.dma_start(qout_d[ro : ro + 128, 0:F], h_sb[half][:, 0:F].bitcast(f32))
                if nsteps > 0:
                    nc.sync.dma_start(qout_d[ro : ro + 128, F : 2 * F], r_sb[half][:, 0:F].bitcast(f32))

    nc.compile()
    return nc


def _get_program(T_pad: int) -> bass.Bass:
    nsteps = int(os.environ.get("KERNEL_NSTEPS", str(STEPS)))
    key = (T_pad, nsteps)
    if key not in _PROG_CACHE:
        _PROG_CACHE[key] = _build_program(T_pad, nsteps)
    return _PROG_CACHE[key]


def make_in_maps(x, batch, cos_coef, q_star, W_ih, W_hh, b_ih, b_hh):
    """Host-side shard + re-layout. Returns (in_maps, T_pad)."""
    x = np.ascontiguousarray(np.asarray(x, dtype=np.float32))
    batch = np.asarray(batch).astype(np.int64)
    cos = np.asarray(cos_coef, dtype=np.float32)
    qs = np.asarray(q_star, dtype=np.float32)
    W_ih = np.asarray(W_ih, dtype=np.float32)
    W_hh = np.asarray(W_hh, dtype=np.float32)
    bsum = (np.asarray(b_ih, dtype=np.float32) + np.asarray(b_hh, dtype=np.float32))

    counts = np.bincount(batch, minlength=B)
    starts = np.zeros(B + 1, dtype=np.int64)
    starts[1:] = np.cumsum(counts)
    blk_counts = counts.reshape(-1, BS).sum(axis=1)
    T_pad = int(max(1, -(-blk_counts.max() // 128)))
    NT = BLOCKS * T_pad
    XW = NT * FW + 56

    z16 = np.zeros((16, 800), dtype=np.float32)
    z24 = np.zeros((24, 800), dtype=np.float32)
    wbig = np.concatenate(
        [W_ih.T, z16, W_hh.T, bsum[None, :]], axis=0
    ).astype(np.float32)  # [617, 800]
    wc = np.concatenate(
        [W_ih[:, :F].T + W_hh.T, z24, W_ih[:, F:].T], axis=0
    ).astype(np.float32)  # [424, 800]

    in_maps = []
    for c in range(CORES):
        seg0 = c * SEG_PER_CORE
        xt = np.zeros((128, XW), dtype=np.float32)
        w = np.zeros((128, NT * BS), dtype=np.float32)
        wt = np.zeros((128, 2 * T_pad * 128), dtype=np.float32)
        cosp = np.zeros((128, NT), dtype=np.float32)
        for g in range(BLOCKS):
            sa = seg0 + g * BS
            n0, n1 = int(starts[sa]), int(starts[sa + BS])
            cnt = n1 - n0
            js = (batch[n0:n1] - sa).astype(np.int64)

            xb = np.zeros((T_pad * 128, FW), dtype=np.float32)
            xb[:cnt, :F] = x[n0:n1]
            xb[:cnt, F] = 1.0
            xt[:, g * T_pad * FW : (g + 1) * T_pad * FW] = (
                xb.reshape(T_pad, 128, FW).transpose(1, 0, 2).reshape(128, T_pad * FW)
            )

            wb = np.zeros((T_pad * 128, BS), dtype=np.float32)
            wb[np.arange(cnt), js] = 1.0
            w[:, g * T_pad * BS : (g + 1) * T_pad * BS] = (
                wb.reshape(T_pad, 128, BS).transpose(1, 0, 2).reshape(128, T_pad * BS)
            )

            cb = np.zeros(T_pad * 128, dtype=np.float32)
            cb[:cnt] = cos[n0:n1]
            cosp[:, g * T_pad : (g + 1) * T_pad] = cb.reshape(T_pad, 128).T

            wb3 = wb.reshape(T_pad, 128, BS)
            lane = g % 4
            for i in range(T_pad):
                slot = (g // 4) * T_pad + i
                wt[32 * lane : 32 * lane + 32, 128 * slot : 128 * slot + 128] = wb3[i].T

        qs0t = np.ones((401, 256), dtype=np.float32)
        qs0t[0:400] = qs[seg0 : seg0 + SEG_PER_CORE].T
        in_maps.append(
            {
                "xt": xt,
                "w": w,
                "wt": wt,
                "cosp": cosp,
                "qs0t": qs0t,
                "ident": np.eye(128, dtype=np.float32),
                "zro": np.zeros((128, 256), dtype=np.float32),
                "wbig": wbig,
                "wc": wc,
            }
        )
    return in_maps, T_pad


def kernel(x, batch, cos_coef, q_star, W_ih, W_hh, b_ih, b_hh):
    global LAST_RESULT
    in_maps, T_pad = make_in_maps(
        x, batch, cos_coef, q_star, W_ih, W_hh, b_ih, b_hh
    )
    nc = _get_program(T_pad)
    res = run_bass_kernel_spmd(nc, in_maps, list(range(CORES)), trace=TRACE)
    LAST_RESULT = res
    out = np.zeros((B, 2 * F), dtype=np.float32)
    for c in range(CORES):
        out[c * SEG_PER_CORE : (c + 1) * SEG_PER_CORE] = res.results[c]["qout"]
    return out
